# revision 3
# baseline (speedup 1.0000x reference)
"""Trainium2 Bass kernel for nn_Decoder (worker/task label-probability decoder).

Math:
    worker_feature = inputs[:2048, :64]          # [Wn, A]
    tau            = inputs[2048:, :16]          # [T, L]
    p1 = sigmoid(x), x = worker_feature @ W + b  # [Wn, 1]
    p2 = (1 - p1) / (L - 1)
    P[i, j, l] = p1[i]^tau[j,l] * p2[i]^(1 - tau[j,l])
               = exp(a[i] * tau[j,l] + c[i])
      with  a = ln(p1/p2) = x + ln(L-1)   (exact: p1/(1-p1) = e^x)
            c = ln p2     = -(x + ln(1 + e^-x)) - ln(L-1)

Sharding: pure data parallel over the worker axis (dim 0), 256 workers per
core across 8 cores; tau/W/b replicated. No communication.

Per-core layout: workers on SBUF partitions (2 groups of 128), flattened
task axis (F = T*L = 32768) streamed in chunks. Each chunk of tau is
replicated to all 128 partitions by a broadcast-AP DMA (partition stride 0)
straight from HBM; the scalar engine then computes Exp(a*tau + c) in one
pass per worker group with per-partition scale/bias, writing bf16 tiles
that stream back to HBM. The rel-err budget (2e-2) comfortably covers the
bf16 output rounding (~2^-9); the host upcasts to fp32. All DMA traffic is
spread round-robin over the SP, DVE and GPSIMD queues so no single engine
serializes the streams; the scalar engine's exp pass is the critical path.
"""

import numpy as np

try:
    import concourse.bass as bass  # noqa: F401
except ImportError:  # fall back to the container's repo checkout
    import sys

    for _p in ("/root/.axon_site/_ro/trn_rl_repo", "/opt/trn_rl_repo"):
        if _p not in sys.path:
            sys.path.append(_p)

import concourse.bass as bass
import concourse.tile as tile
from concourse import mybir
from concourse.bass_utils import run_bass_kernel_spmd

WN = 2048  # workers total
TN = 2048  # tasks
L = 16  # edge types / labels
A = 64  # ability features
NCORES = 8
WPC = WN // NCORES  # workers per core (256)
G = WPC // 128  # partition groups per core (2)
F = TN * L  # flattened task axis (32768)

LN15 = float(np.log(np.float32(L - 1)).astype(np.float32))

# Chunk schedule over the F axis: (size, n_rep_sub_dmas, n_out_sub_dmas).
# Small chunks at the ends keep the pipeline ramp and tail short; large
# chunks in the middle amortize the per-op ACT overhead. Sizes come in two
# flavors (S and XL) so tile pools stay shape-uniform per tag.
SZ_S = 2048
SZ_L = 8192
CHUNKS = (
    [(SZ_S, 1, 1)] * 2
    + [(SZ_L, 2, 2)] * 3
    + [(SZ_S, 1, 1)] * 2
)
assert sum(c[0] for c in CHUNKS) == F

_AF = mybir.ActivationFunctionType


class _TC(tile.TileContext):
    """TileContext legalized for a walrus that allows one sync-wait per inst.

    The walrus build in this container rejects any instruction carrying more
    than one sync-wait command. After Tile's normal scheduling + the exit
    drain/barrier, rewrite every multi-wait instruction into a chain of
    same-engine NOPs (one wait each) followed by the instruction with the
    final wait.
    """

    def _drain_and_barrier(self, tick_clock, wait_clock):
        super()._drain_and_barrier(tick_clock, wait_clock)
        self._split_multi_waits()

    def _fresh_nop(self, engine):
        inst = self.nc.engines[engine].nop(nofuse=True).ins
        self.nc.cur_bb.bb.instructions.remove(inst)
        return inst

    def _split_multi_waits(self):
        for fn in self.nc.m.functions:
            for bb in fn.blocks:
                snapshot = list(bb.instructions)
                if not any(
                    inst.sync_info and len(inst.sync_info.on_wait) > 1
                    for inst in snapshot
                ):
                    continue
                new = []
                for inst in snapshot:
                    si = inst.sync_info
                    if si is not None and si.on_wait and len(si.on_wait) > 1:
                        waits = list(si.on_wait)
                        si.on_wait = waits[-1:]
                        inst.sync_info = si
                        for wt in waits[:-1]:
                            nop = self._fresh_nop(inst.engine)
                            nop.sync_info = mybir.SyncInfo(on_wait=[wt], on_update=[])
                            new.append(nop)
                    new.append(inst)
                bb.instructions[:] = new


def build_nc():
    nc = bass.Bass("TRN2")
    wf = nc.dram_tensor("wf", [WPC, A], mybir.dt.float32, kind="ExternalInput")
    tau_in = nc.dram_tensor("tau", [F], mybir.dt.float32, kind="ExternalInput")
    w_in = nc.dram_tensor("W", [A], mybir.dt.float32, kind="ExternalInput")
    b_in = nc.dram_tensor("b", [1], mybir.dt.float32, kind="ExternalInput")
    out = nc.dram_tensor("out", [G, 128, F], mybir.dt.bfloat16, kind="ExternalOutput")

    f32 = mybir.dt.float32
    bf16 = mybir.dt.bfloat16

    # Round-robin DMA issuers: each engine is an independent DMA channel.
    # Only SP and GPSIMD may issue DMAs without touching the (bottleneck)
    # scalar engine.
    def rep_engine(i):
        return [nc.sync, nc.gpsimd][i % 2]

    def out_engine(i):
        return [nc.gpsimd, nc.sync][i % 2]

    with _TC(nc) as tc:
        with (
            tc.tile_pool(name="const", bufs=1) as const,
            tc.tile_pool(name="main", bufs=2) as main,
        ):
            # ---- warm the ACT exp/ln table before anything depends on it ----
            dummy = const.tile([128, 1], f32)
            nc.vector.memset(dummy, 0.0)
            dummy2 = const.tile([128, 1], f32)
            nc.scalar.activation(dummy2, dummy, _AF.Exp)

            # ---- constant loads ----
            wf_sb = const.tile([128, G, A], f32)
            nc.sync.dma_start(
                out=wf_sb, in_=wf[:].rearrange("(g p) a -> p g a", p=128)
            )
            w_ap = w_in[:]
            w_sb = const.tile([128, A], f32)
            nc.sync.dma_start(
                out=w_sb,
                in_=bass.AP(tensor=w_ap.tensor, offset=w_ap.offset, ap=[[0, 128], [1, A]]),
            )
            b_ap = b_in[:]
            b_sb = const.tile([128, 1], f32)
            nc.sync.dma_start(
                out=b_sb,
                in_=bass.AP(tensor=b_ap.tensor, offset=b_ap.offset, ap=[[0, 128], [1, 1]]),
            )

            # ---- per-worker scalars: a = x + ln15, c = -(x + b + ln(1+e^-(x+b))) - ln15
            x = const.tile([128, G], f32)
            for g in range(G):
                prod = const.tile([128, A], f32, tag=f"prod{g}")
                nc.vector.tensor_mul(prod, wf_sb[:, g, :], w_sb)
                nc.vector.reduce_sum(x[:, g : g + 1], prod, axis=mybir.AxisListType.X)

            xb = const.tile([128, G], f32)
            nc.scalar.activation(xb, x, _AF.Identity, bias=b_sb[:, 0:1])
            e = const.tile([128, G], f32)
            nc.scalar.activation(e, xb, _AF.Exp, scale=-1.0)
            s = const.tile([128, G], f32)
            nc.vector.tensor_scalar_add(s, e, 1.0)
            ls = const.tile([128, G], f32)
            nc.scalar.activation(ls, s, _AF.Ln)
            u = const.tile([128, G], f32)
            nc.vector.tensor_add(u, xb, ls)
            c_sb = const.tile([128, G], f32)
            nc.vector.tensor_scalar(
                c_sb,
                u,
                scalar1=-1.0,
                scalar2=-LN15,
                op0=mybir.AluOpType.mult,
                op1=mybir.AluOpType.add,
            )
            a_sb = const.tile([128, G], f32)
            nc.vector.tensor_scalar_add(a_sb, xb, LN15)

            # ---- main loop: broadcast-replicate tau chunk -> ACT exp -> DMA out
            tau_ap = tau_in[:]
            dma_i = 0
            out_i = 0
            f0 = 0
            for k, (sz, nrep, nout) in enumerate(CHUNKS):
                tag = "s" if sz == SZ_S else "l"
                rep = main.tile([128, sz], f32, tag=f"rep_{tag}", name=f"rep{k}", bufs=2)
                sub = sz // nrep
                for j in range(nrep):
                    rep_engine(dma_i).dma_start(
                        out=rep[:, j * sub : (j + 1) * sub],
                        in_=bass.AP(
                            tensor=tau_ap.tensor,
                            offset=tau_ap.offset + f0 + j * sub,
                            ap=[[0, 128], [1, sub]],
                        ),
                    )
                    dma_i += 1
                for g in range(G):
                    ot = main.tile(
                        [128, sz], bf16, tag=f"ot_{tag}{g}", name=f"ot{k}g{g}", bufs=2
                    )
                    nc.scalar.activation(
                        ot,
                        rep,
                        _AF.Exp,
                        bias=c_sb[:, g : g + 1],
                        scale=a_sb[:, g : g + 1],
                    )
                    osub = sz // nout
                    for j in range(nout):
                        out_engine(out_i).dma_start(
                            out=out[g, :, f0 + j * osub : f0 + (j + 1) * osub],
                            in_=ot[:, j * osub : (j + 1) * osub],
                        )
                        out_i += 1
                f0 += sz
    return nc


_NC = None


def kernel(inputs, W, b, worker_num=WN, task_num=TN, edge_type=L, ability_num=A, **_kw):
    global _NC
    inputs = np.ascontiguousarray(np.asarray(inputs, dtype=np.float32))
    W = np.asarray(W, dtype=np.float32).reshape(A)
    b = np.asarray(b, dtype=np.float32).reshape(1)
    assert inputs.shape == (WN + TN, A)

    wf = inputs[:WN, :A]
    tau = np.ascontiguousarray(inputs[WN:, :L].reshape(F))

    if _NC is None:
        _NC = build_nc()

    in_maps = [
        {
            "wf": np.ascontiguousarray(wf[k * WPC : (k + 1) * WPC]),
            "tau": tau,
            "W": W,
            "b": b,
        }
        for k in range(NCORES)
    ]
    res = run_bass_kernel_spmd(_NC, in_maps, core_ids=list(range(NCORES)))
    parts = [
        np.asarray(r["out"]).astype(np.float32).reshape(WPC, TN, L)
        for r in res.results
    ]
    return np.concatenate(parts, axis=0)


# revision 6
# speedup vs baseline: 1.0494x; 1.0494x over previous
"""Trainium2 Bass kernel for nn_Decoder (worker/task label-probability decoder).

Math:
    worker_feature = inputs[:2048, :64]          # [Wn, A]
    tau            = inputs[2048:, :16]          # [T, L]
    p1 = sigmoid(x), x = worker_feature @ W + b  # [Wn, 1]
    p2 = (1 - p1) / (L - 1)
    P[i, j, l] = p1[i]^tau[j,l] * p2[i]^(1 - tau[j,l])
               = exp(a[i] * tau[j,l] + c[i])
      with  a = ln(p1/p2) = x + ln(L-1)   (exact: p1/(1-p1) = e^x)
            c = ln p2     = -(x + ln(1 + e^-x)) - ln(L-1)

Sharding: pure data parallel over the worker axis (dim 0), 256 workers per
core across 8 cores; tau/W/b replicated. No communication.

Per-core layout: workers on SBUF partitions (2 groups of 128), flattened
task axis (F = T*L = 32768) streamed in chunks. Each chunk of tau is
replicated to all 128 partitions by a broadcast-AP DMA (partition stride 0)
straight from HBM; the scalar engine then computes Exp(a*tau + c) in one
pass per worker group with per-partition scale/bias, writing bf16 tiles
that stream back to HBM. The rel-err budget (2e-2) comfortably covers the
bf16 output rounding (~2^-9); the host upcasts to fp32. All DMA traffic is
spread round-robin over the SP, DVE and GPSIMD queues so no single engine
serializes the streams; the scalar engine's exp pass is the critical path.
"""

import numpy as np

try:
    import concourse.bass as bass  # noqa: F401
except ImportError:  # fall back to the container's repo checkout
    import sys

    for _p in ("/root/.axon_site/_ro/trn_rl_repo", "/opt/trn_rl_repo"):
        if _p not in sys.path:
            sys.path.append(_p)

import concourse.bass as bass
import concourse.tile as tile
from concourse import mybir
from concourse.bass_utils import run_bass_kernel_spmd

WN = 2048  # workers total
TN = 2048  # tasks
L = 16  # edge types / labels
A = 64  # ability features
NCORES = 8
WPC = WN // NCORES  # workers per core (256)
G = WPC // 128  # partition groups per core (2)
F = TN * L  # flattened task axis (32768)

LN15 = float(np.log(np.float32(L - 1)).astype(np.float32))

# Chunk schedule over the F axis: (size, n_rep_sub_dmas, n_out_sub_dmas).
# Small chunks at the ends keep the pipeline ramp and tail short; large
# chunks in the middle amortize the per-op ACT overhead. Sizes come in two
# flavors (S and XL) so tile pools stay shape-uniform per tag.
SZ_S = 2048
SZ_L = 8192
CHUNKS = (
    [(SZ_S, 1, 1)] * 2
    + [(SZ_L, 2, 2)] * 3
    + [(SZ_S, 1, 1)] + [(SZ_S, 1, 2)]
)
assert sum(c[0] for c in CHUNKS) == F

_AF = mybir.ActivationFunctionType


class _TC(tile.TileContext):
    """TileContext legalized for a walrus that allows one sync-wait per inst.

    The walrus build in this container rejects any instruction carrying more
    than one sync-wait command. After Tile's normal scheduling + the exit
    drain/barrier, rewrite every multi-wait instruction into a chain of
    same-engine NOPs (one wait each) followed by the instruction with the
    final wait.
    """

    def _drain_and_barrier(self, tick_clock, wait_clock):
        super()._drain_and_barrier(tick_clock, wait_clock)
        self._split_multi_waits()

    def _fresh_nop(self, engine):
        inst = self.nc.engines[engine].nop(nofuse=True).ins
        self.nc.cur_bb.bb.instructions.remove(inst)
        return inst

    def _split_multi_waits(self):
        for fn in self.nc.m.functions:
            for bb in fn.blocks:
                snapshot = list(bb.instructions)
                if not any(
                    inst.sync_info and len(inst.sync_info.on_wait) > 1
                    for inst in snapshot
                ):
                    continue
                new = []
                for inst in snapshot:
                    si = inst.sync_info
                    if si is not None and si.on_wait and len(si.on_wait) > 1:
                        waits = list(si.on_wait)
                        si.on_wait = waits[-1:]
                        inst.sync_info = si
                        for wt in waits[:-1]:
                            nop = self._fresh_nop(inst.engine)
                            nop.sync_info = mybir.SyncInfo(on_wait=[wt], on_update=[])
                            new.append(nop)
                    new.append(inst)
                bb.instructions[:] = new


def build_nc():
    nc = bass.Bass("TRN2")
    wf = nc.dram_tensor("wf", [WPC, A], mybir.dt.float32, kind="ExternalInput")
    tau_in = nc.dram_tensor("tau", [F], mybir.dt.float32, kind="ExternalInput")
    w_in = nc.dram_tensor("W", [A], mybir.dt.float32, kind="ExternalInput")
    b_in = nc.dram_tensor("b", [1], mybir.dt.float32, kind="ExternalInput")
    out = nc.dram_tensor("out", [G, 128, F], mybir.dt.bfloat16, kind="ExternalOutput")

    f32 = mybir.dt.float32
    bf16 = mybir.dt.bfloat16

    # Round-robin DMA issuers: each engine is an independent DMA channel.
    # Only SP and GPSIMD may issue DMAs without touching the (bottleneck)
    # scalar engine.
    def rep_engine(i):
        return [nc.sync, nc.gpsimd][i % 2]

    def out_engine(i):
        return [nc.gpsimd, nc.sync][i % 2]

    with _TC(nc) as tc:
        with (
            tc.tile_pool(name="const", bufs=1) as const,
            tc.tile_pool(name="main", bufs=2) as main,
        ):
            # ---- warm the ACT exp/ln table before anything depends on it ----
            dummy = const.tile([128, 1], f32)
            nc.vector.memset(dummy, 0.0)
            dummy2 = const.tile([128, 1], f32)
            nc.scalar.activation(dummy2, dummy, _AF.Exp)

            # ---- first tau chunk on the otherwise-idle GPSIMD queue, at t=0,
            # so the scalar engine can start the moment a/c are ready ----
            tau_ap = tau_in[:]
            sz0 = CHUNKS[0][0]
            rep0 = main.tile([128, sz0], f32, tag="rep_s", name="rep0", bufs=2)
            nc.gpsimd.dma_start(
                out=rep0,
                in_=bass.AP(
                    tensor=tau_ap.tensor, offset=tau_ap.offset, ap=[[0, 128], [1, sz0]]
                ),
            )

            # ---- constant loads ----
            wf_sb = const.tile([128, G, A], f32)
            nc.sync.dma_start(
                out=wf_sb, in_=wf[:].rearrange("(g p) a -> p g a", p=128)
            )
            w_ap = w_in[:]
            w_sb = const.tile([128, A], f32)
            nc.sync.dma_start(
                out=w_sb,
                in_=bass.AP(tensor=w_ap.tensor, offset=w_ap.offset, ap=[[0, 128], [1, A]]),
            )
            b_ap = b_in[:]
            b_sb = const.tile([128, 1], f32)
            nc.sync.dma_start(
                out=b_sb,
                in_=bass.AP(tensor=b_ap.tensor, offset=b_ap.offset, ap=[[0, 128], [1, 1]]),
            )

            # ---- per-worker scalars: a = x + ln15, c = -(x + b + ln(1+e^-(x+b))) - ln15
            x = const.tile([128, G], f32)
            for g in range(G):
                prod = const.tile([128, A], f32, tag=f"prod{g}")
                nc.vector.tensor_mul(prod, wf_sb[:, g, :], w_sb)
                nc.vector.reduce_sum(x[:, g : g + 1], prod, axis=mybir.AxisListType.X)

            xb = const.tile([128, G], f32)
            nc.scalar.activation(xb, x, _AF.Identity, bias=b_sb[:, 0:1])
            e = const.tile([128, G], f32)
            nc.scalar.activation(e, xb, _AF.Exp, scale=-1.0)
            s = const.tile([128, G], f32)
            nc.vector.tensor_scalar_add(s, e, 1.0)
            ls = const.tile([128, G], f32)
            nc.scalar.activation(ls, s, _AF.Ln)
            u = const.tile([128, G], f32)
            nc.vector.tensor_add(u, xb, ls)
            c_sb = const.tile([128, G], f32)
            nc.vector.tensor_scalar(
                c_sb,
                u,
                scalar1=-1.0,
                scalar2=-LN15,
                op0=mybir.AluOpType.mult,
                op1=mybir.AluOpType.add,
            )
            a_sb = const.tile([128, G], f32)
            nc.vector.tensor_scalar_add(a_sb, xb, LN15)

            # ---- main loop: broadcast-replicate tau chunk -> ACT exp -> DMA out
            dma_i = 0
            out_i = 0
            f0 = 0
            for k, (sz, nrep, nout) in enumerate(CHUNKS):
                tag = "s" if sz == SZ_S else "l"
                if k == 0:
                    rep = rep0
                else:
                    rep = main.tile(
                        [128, sz], f32, tag=f"rep_{tag}", name=f"rep{k}", bufs=2
                    )
                    sub = sz // nrep
                    for j in range(nrep):
                        rep_engine(dma_i).dma_start(
                            out=rep[:, j * sub : (j + 1) * sub],
                            in_=bass.AP(
                                tensor=tau_ap.tensor,
                                offset=tau_ap.offset + f0 + j * sub,
                                ap=[[0, 128], [1, sub]],
                            ),
                        )
                        dma_i += 1
                for g in range(G):
                    ot = main.tile(
                        [128, sz], bf16, tag=f"ot_{tag}{g}", name=f"ot{k}g{g}", bufs=2
                    )
                    nc.scalar.activation(
                        ot,
                        rep,
                        _AF.Exp,
                        bias=c_sb[:, g : g + 1],
                        scale=a_sb[:, g : g + 1],
                    )
                    osub = sz // nout
                    for j in range(nout):
                        out_engine(out_i).dma_start(
                            out=out[g, :, f0 + j * osub : f0 + (j + 1) * osub],
                            in_=ot[:, j * osub : (j + 1) * osub],
                        )
                        out_i += 1
                f0 += sz
    return nc


_NC = None


def kernel(inputs, W, b, worker_num=WN, task_num=TN, edge_type=L, ability_num=A, **_kw):
    global _NC
    inputs = np.ascontiguousarray(np.asarray(inputs, dtype=np.float32))
    W = np.asarray(W, dtype=np.float32).reshape(A)
    b = np.asarray(b, dtype=np.float32).reshape(1)
    assert inputs.shape == (WN + TN, A)

    wf = inputs[:WN, :A]
    tau = np.ascontiguousarray(inputs[WN:, :L].reshape(F))

    if _NC is None:
        _NC = build_nc()

    in_maps = [
        {
            "wf": np.ascontiguousarray(wf[k * WPC : (k + 1) * WPC]),
            "tau": tau,
            "W": W,
            "b": b,
        }
        for k in range(NCORES)
    ]
    res = run_bass_kernel_spmd(_NC, in_maps, core_ids=list(range(NCORES)))
    parts = [
        np.asarray(r["out"]).astype(np.float32).reshape(WPC, TN, L)
        for r in res.results
    ]
    return np.concatenate(parts, axis=0)


# revision 9
# speedup vs baseline: 1.1422x; 1.0885x over previous
"""Trainium2 Bass kernel for nn_Decoder (worker/task label-probability decoder).

Math:
    worker_feature = inputs[:2048, :64]          # [Wn, A]
    tau            = inputs[2048:, :16]          # [T, L]
    p1 = sigmoid(x), x = worker_feature @ W + b  # [Wn, 1]
    p2 = (1 - p1) / (L - 1)
    P[i, j, l] = p1[i]^tau[j,l] * p2[i]^(1 - tau[j,l])
               = exp(a[i] * tau[j,l] + c[i])
      with  a = ln(p1/p2) = x + ln(L-1)   (exact: p1/(1-p1) = e^x)
            c = ln p2     = -(x + ln(1 + e^-x)) - ln(L-1)

Sharding: pure data parallel over the worker axis (dim 0), 256 workers per
core across 8 cores; tau/W/b replicated. No communication.

Per-core layout: workers on SBUF partitions (2 groups of 128), flattened
task axis (F = T*L = 32768) streamed in chunks. Each chunk of tau is
replicated to all 128 partitions by a broadcast-AP DMA (partition stride 0)
straight from HBM; the scalar engine then computes Exp(a*tau + c) in one
pass per worker group with per-partition scale/bias, writing bf16 tiles
that stream back to HBM. The rel-err budget (2e-2) comfortably covers the
bf16 output rounding (~2^-9); the host upcasts to fp32. All DMA traffic is
spread round-robin over the SP, DVE and GPSIMD queues so no single engine
serializes the streams; the scalar engine's exp pass is the critical path.
"""

import numpy as np

try:
    import concourse.bass as bass  # noqa: F401
except ImportError:  # fall back to the container's repo checkout
    import sys

    for _p in ("/root/.axon_site/_ro/trn_rl_repo", "/opt/trn_rl_repo"):
        if _p not in sys.path:
            sys.path.append(_p)

import concourse.bass as bass
import concourse.tile as tile
from concourse import mybir
from concourse.bass_utils import run_bass_kernel_spmd

WN = 2048  # workers total
TN = 2048  # tasks
L = 16  # edge types / labels
A = 64  # ability features
NCORES = 8
WPC = WN // NCORES  # workers per core (256)
G = WPC // 128  # partition groups per core (2)
F = TN * L  # flattened task axis (32768)

LN15 = float(np.log(np.float32(L - 1)).astype(np.float32))

# Chunk schedule over the F axis: (size, n_rep_sub_dmas, n_out_sub_dmas).
# Small chunks at the ends keep the pipeline ramp and tail short; large
# chunks in the middle amortize the per-op ACT overhead. Sizes come in two
# flavors (S and XL) so tile pools stay shape-uniform per tag.
SZ_S = 2048
SZ_L = 8192
# (size, n_rep_sub_dmas, n_out_sub_dmas, dve_cols): dve_cols trailing columns
# of the chunk (per worker group) are computed by the vector engine's
# bitcast-exp2 pipeline instead of ACT, shifting work off the critical path.
DVE_D = 1024
CHUNKS = (
    [(SZ_S, 1, 1, 0)] * 2
    + [(SZ_L, 2, 2, DVE_D)] * 3
    + [(SZ_S, 1, 1, 0)] + [(SZ_S, 1, 2, 0)]
)
assert sum(c[0] for c in CHUNKS) == F

# Bitcast exp2: for t = z*log2(e) in (-127, 0], let y = int32(t*2^23 +
# 127*2^23). Bitcasting y to f32 gives s = 2^t * (1+f)/2^f where f is the
# fractional part actually encoded in y's mantissa. Correct multiplicatively
# with g(f) = 2^f/(1+f) evaluated as a degree-2 minimax polynomial of the
# mantissa integer m = y & 0x7fffff (exact in f32). Max rel err ~6.4e-3.
EXP_SCALE = float(np.log2(np.e) * (1 << 23))
EXP_BIAS = float(127.0 * (1 << 23))
Q2 = 0.22573194345762757 / (1 << 23) ** 2
Q1 = -0.2151853848831074 / (1 << 23)
Q0 = 0.993559438904892
MANT_MASK = 0x007FFFFF

_AF = mybir.ActivationFunctionType


class _TC(tile.TileContext):
    """TileContext legalized for a walrus that allows one sync-wait per inst.

    The walrus build in this container rejects any instruction carrying more
    than one sync-wait command. After Tile's normal scheduling + the exit
    drain/barrier, rewrite every multi-wait instruction into a chain of
    same-engine NOPs (one wait each) followed by the instruction with the
    final wait.
    """

    def _drain_and_barrier(self, tick_clock, wait_clock):
        super()._drain_and_barrier(tick_clock, wait_clock)
        self._split_multi_waits()

    def _fresh_nop(self, engine):
        inst = self.nc.engines[engine].nop(nofuse=True).ins
        self.nc.cur_bb.bb.instructions.remove(inst)
        return inst

    def _split_multi_waits(self):
        for fn in self.nc.m.functions:
            for bb in fn.blocks:
                snapshot = list(bb.instructions)
                if not any(
                    inst.sync_info and len(inst.sync_info.on_wait) > 1
                    for inst in snapshot
                ):
                    continue
                new = []
                for inst in snapshot:
                    si = inst.sync_info
                    if si is not None and si.on_wait and len(si.on_wait) > 1:
                        waits = list(si.on_wait)
                        si.on_wait = waits[-1:]
                        inst.sync_info = si
                        for wt in waits[:-1]:
                            nop = self._fresh_nop(inst.engine)
                            nop.sync_info = mybir.SyncInfo(on_wait=[wt], on_update=[])
                            new.append(nop)
                    new.append(inst)
                bb.instructions[:] = new


def build_nc():
    nc = bass.Bass("TRN2")
    wf = nc.dram_tensor("wf", [WPC, A], mybir.dt.float32, kind="ExternalInput")
    tau_in = nc.dram_tensor("tau", [F], mybir.dt.float32, kind="ExternalInput")
    w_in = nc.dram_tensor("W", [A], mybir.dt.float32, kind="ExternalInput")
    b_in = nc.dram_tensor("b", [1], mybir.dt.float32, kind="ExternalInput")
    out = nc.dram_tensor("out", [G, 128, F], mybir.dt.bfloat16, kind="ExternalOutput")

    f32 = mybir.dt.float32
    bf16 = mybir.dt.bfloat16

    # Round-robin DMA issuers: each engine is an independent DMA channel.
    # Only SP and GPSIMD may issue DMAs without touching the (bottleneck)
    # scalar engine.
    def rep_engine(i):
        return [nc.sync, nc.gpsimd][i % 2]

    def out_engine(i):
        return [nc.gpsimd, nc.sync][i % 2]

    with _TC(nc) as tc:
        with (
            tc.tile_pool(name="const", bufs=1) as const,
            tc.tile_pool(name="main", bufs=2) as main,
        ):
            # ---- warm the ACT exp/ln table before anything depends on it ----
            dummy = const.tile([128, 1], f32)
            nc.vector.memset(dummy, 0.0)
            dummy2 = const.tile([128, 1], f32)
            nc.scalar.activation(dummy2, dummy, _AF.Exp)

            # ---- first tau chunk on the otherwise-idle GPSIMD queue, at t=0,
            # so the scalar engine can start the moment a/c are ready ----
            tau_ap = tau_in[:]
            sz0 = CHUNKS[0][0]
            rep0 = main.tile([128, sz0], f32, tag="rep_s", name="rep0", bufs=2)
            nc.gpsimd.dma_start(
                out=rep0,
                in_=bass.AP(
                    tensor=tau_ap.tensor, offset=tau_ap.offset, ap=[[0, 128], [1, sz0]]
                ),
            )

            # ---- constant loads ----
            wf_sb = const.tile([128, G, A], f32)
            nc.sync.dma_start(
                out=wf_sb, in_=wf[:].rearrange("(g p) a -> p g a", p=128)
            )
            w_ap = w_in[:]
            w_sb = const.tile([128, A], f32)
            nc.sync.dma_start(
                out=w_sb,
                in_=bass.AP(tensor=w_ap.tensor, offset=w_ap.offset, ap=[[0, 128], [1, A]]),
            )
            b_ap = b_in[:]
            b_sb = const.tile([128, 1], f32)
            nc.sync.dma_start(
                out=b_sb,
                in_=bass.AP(tensor=b_ap.tensor, offset=b_ap.offset, ap=[[0, 128], [1, 1]]),
            )

            # ---- per-worker scalars: a = x + ln15, c = -(x + b + ln(1+e^-(x+b))) - ln15
            x = const.tile([128, G], f32)
            for g in range(G):
                prod = const.tile([128, A], f32, tag=f"prod{g}")
                nc.vector.tensor_mul(prod, wf_sb[:, g, :], w_sb)
                nc.vector.reduce_sum(x[:, g : g + 1], prod, axis=mybir.AxisListType.X)

            xb = const.tile([128, G], f32)
            nc.scalar.activation(xb, x, _AF.Identity, bias=b_sb[:, 0:1])
            e = const.tile([128, G], f32)
            nc.scalar.activation(e, xb, _AF.Exp, scale=-1.0)
            s = const.tile([128, G], f32)
            nc.vector.tensor_scalar_add(s, e, 1.0)
            ls = const.tile([128, G], f32)
            nc.scalar.activation(ls, s, _AF.Ln)
            u = const.tile([128, G], f32)
            nc.vector.tensor_add(u, xb, ls)
            c_sb = const.tile([128, G], f32)
            nc.vector.tensor_scalar(
                c_sb,
                u,
                scalar1=-1.0,
                scalar2=-LN15,
                op0=mybir.AluOpType.mult,
                op1=mybir.AluOpType.add,
            )
            a_sb = const.tile([128, G], f32)
            nc.vector.tensor_scalar_add(a_sb, xb, LN15)

            # scaled affine constants for the DVE bitcast-exp2 path:
            #   y = a_scaled * tau + ccorr  ->  int32  (Schraudolph biasing)
            a_sc = const.tile([128, G], f32)
            nc.vector.tensor_scalar_mul(a_sc, a_sb, EXP_SCALE)
            ccorr = const.tile([128, G], f32)
            nc.vector.tensor_scalar(
                ccorr,
                c_sb,
                scalar1=EXP_SCALE,
                scalar2=EXP_BIAS,
                op0=mybir.AluOpType.mult,
                op1=mybir.AluOpType.add,
            )

            # ---- main loop: broadcast-replicate tau chunk -> exp -> DMA out.
            # ACT computes most columns; the otherwise-idle DVE handles the
            # trailing dve_cols of the big chunks via bitcast-exp2.
            i32 = mybir.dt.int32

            def dve_exp(ot_slice, rep_slice, g, key):
                d = rep_slice.shape[-1]
                y = main.tile([128, d], i32, tag="dve_y", name=f"y{key}", bufs=1)
                nc.vector.tensor_scalar(
                    y,
                    rep_slice,
                    scalar1=a_sc[:, g : g + 1],
                    scalar2=ccorr[:, g : g + 1],
                    op0=mybir.AluOpType.mult,
                    op1=mybir.AluOpType.add,
                )
                m = main.tile([128, d], i32, tag="dve_m", name=f"m{key}", bufs=1)
                nc.vector.tensor_scalar(
                    m, y, scalar1=MANT_MASK, scalar2=None, op0=mybir.AluOpType.bitwise_and
                )
                fm = main.tile([128, d], f32, tag="dve_fm", name=f"fm{key}", bufs=1)
                nc.vector.tensor_scalar_add(fm, m, 0.0)
                h1 = main.tile([128, d], f32, tag="dve_h1", name=f"h1{key}", bufs=1)
                nc.vector.tensor_scalar(
                    h1,
                    fm,
                    scalar1=Q2,
                    scalar2=Q1,
                    op0=mybir.AluOpType.mult,
                    op1=mybir.AluOpType.add,
                )
                h2 = main.tile([128, d], f32, tag="dve_h2", name=f"h2{key}", bufs=1)
                nc.vector.tensor_mul(h2, h1, fm)
                h3 = main.tile([128, d], f32, tag="dve_h3", name=f"h3{key}", bufs=1)
                nc.vector.tensor_scalar_add(h3, h2, Q0)
                nc.vector.tensor_mul(ot_slice, h3, y.bitcast(f32))

            dma_i = 0
            out_i = 0
            f0 = 0
            for k, (sz, nrep, nout, dve) in enumerate(CHUNKS):
                tag = "s" if sz == SZ_S else "l"
                if k == 0:
                    rep = rep0
                else:
                    rep = main.tile(
                        [128, sz], f32, tag=f"rep_{tag}", name=f"rep{k}", bufs=2
                    )
                    sub = sz // nrep
                    for j in range(nrep):
                        rep_engine(dma_i).dma_start(
                            out=rep[:, j * sub : (j + 1) * sub],
                            in_=bass.AP(
                                tensor=tau_ap.tensor,
                                offset=tau_ap.offset + f0 + j * sub,
                                ap=[[0, 128], [1, sub]],
                            ),
                        )
                        dma_i += 1
                act_cols = sz - dve
                for g in range(G):
                    ot = main.tile(
                        [128, sz], bf16, tag=f"ot_{tag}{g}", name=f"ot{k}g{g}", bufs=2
                    )
                    nc.scalar.activation(
                        ot[:, :act_cols],
                        rep[:, :act_cols],
                        _AF.Exp,
                        bias=c_sb[:, g : g + 1],
                        scale=a_sb[:, g : g + 1],
                    )
                    if dve:
                        dve_exp(ot[:, act_cols:], rep[:, act_cols:], g, f"{k}g{g}")
                    osub = sz // nout
                    for j in range(nout):
                        out_engine(out_i).dma_start(
                            out=out[g, :, f0 + j * osub : f0 + (j + 1) * osub],
                            in_=ot[:, j * osub : (j + 1) * osub],
                        )
                        out_i += 1
                f0 += sz
    return nc


_NC = None


def kernel(inputs, W, b, worker_num=WN, task_num=TN, edge_type=L, ability_num=A, **_kw):
    global _NC
    inputs = np.ascontiguousarray(np.asarray(inputs, dtype=np.float32))
    W = np.asarray(W, dtype=np.float32).reshape(A)
    b = np.asarray(b, dtype=np.float32).reshape(1)
    assert inputs.shape == (WN + TN, A)

    wf = inputs[:WN, :A]
    tau = np.ascontiguousarray(inputs[WN:, :L].reshape(F))

    if _NC is None:
        _NC = build_nc()

    in_maps = [
        {
            "wf": np.ascontiguousarray(wf[k * WPC : (k + 1) * WPC]),
            "tau": tau,
            "W": W,
            "b": b,
        }
        for k in range(NCORES)
    ]
    res = run_bass_kernel_spmd(_NC, in_maps, core_ids=list(range(NCORES)))
    parts = [
        np.asarray(r["out"]).astype(np.float32).reshape(WPC, TN, L)
        for r in res.results
    ]
    return np.concatenate(parts, axis=0)
